# revision 1
# baseline (speedup 1.0000x reference)
"""Fused causal attention block (qkv proj + RoPE + attention + out proj) on 8 TRN2 cores.

Sharding: data-parallel over batch (2) x tensor-parallel over heads (16 -> 4 per core).
Each core computes y_partial[b] = attn_heads_group(x[b]) @ out_w[group_rows]; the host
sums the 4 partials per batch (the out-projection "all-reduce") and stacks batches.

Device kernel (per core, Tile framework):
  phase A: load xT/weights, compute qT,kT (transposed [d_head, s] layout) and v (natural),
           RoPE applied to q,k in transposed layout via paired partition-slice DVE ops.
  phase B: per head, per 512-wide q chunk: scores^T = k_tile^T @ q (PSUM, fp32r),
           causal mask added via identity-matmul bias injection, exp on ACT (scale=1/8,
           no max subtraction -- logits are O(5) by construction), PV accumulation with a
           ones-column appended to v so the softmax denominator falls out of the same
           matmul, normalization via K=1 broadcast matmul + DVE multiply.
  phase C: y = attnT^T @ wo per 128-row s-tile, DMA out.
"""

import numpy as np

S = 2048
D = 1024
H = 16
DH = 64
P = 128
HPC = 4          # heads per core
QC = 512         # q-chunk width
NQC = S // QC
NKT = S // P     # k tiles
DIN_T = D // P   # contraction tiles for projections
NST = S // P     # s tiles
MBIG = -240000.0  # pre-scale mask bias; * 0.125 = -30000 -> exp == 0.0


def _build_nc(is_causal: bool, use_kbias: bool):
    import concourse.bass as bass
    import concourse.mybir as mybir
    import concourse.tile as tile

    f32 = mybir.dt.float32
    f32r = mybir.dt.float32r
    EXP = mybir.ActivationFunctionType.Exp

    nc = bass.Bass()
    wfix_sem = nc.alloc_semaphore("wfix")
    xT = nc.dram_tensor("xT", [D, S], f32r, kind="ExternalInput")
    wq = nc.dram_tensor("wq", [D, 256], f32r, kind="ExternalInput")
    wk = nc.dram_tensor("wk", [D, 256], f32r, kind="ExternalInput")
    wv = nc.dram_tensor("wv", [D, 256], f32r, kind="ExternalInput")
    wo = nc.dram_tensor("wo", [HPC, 64, D], f32r, kind="ExternalInput")
    ctab = nc.dram_tensor("ctab", [P, S], mybir.dt.float16, kind="ExternalInput")
    ttab = nc.dram_tensor("ttab", [P, S], mybir.dt.float16, kind="ExternalInput")
    mask = nc.dram_tensor("mask", [P, 896], f32r, kind="ExternalInput")
    ident = nc.dram_tensor("ident", [P, P], f32r, kind="ExternalInput")
    kbias = nc.dram_tensor("kbias", [1, S], f32r, kind="ExternalInput")
    ones_in = nc.dram_tensor("ones_in", [65, 64], f32r, kind="ExternalInput")
    vones = nc.dram_tensor("vones", [P, NKT], f32r, kind="ExternalInput")
    y = nc.dram_tensor("y", [S, D], f32, kind="ExternalOutput")

    with tile.TileContext(nc) as tc, nc.allow_low_precision(
        reason="fp32r is bit-identical to fp32 here; matmul inputs must be typed fp32r"
    ):
        with (
            tc.tile_pool(name="pers", bufs=1) as pers,
            tc.tile_pool(name="ropet", bufs=2) as ropet,
        ):
            qT_sb = pers.tile([P, 2, S], f32r, tag="qT")
            kT_sb = pers.tile([P, 2, S], f32r, tag="kT")
            v_sb = pers.tile([P, HPC, NKT, 65], f32r, tag="v")
            attnT_sb = pers.tile([64, HPC, S], f32r, tag="attnT")
            wo_sb = pers.tile([64, HPC, D], f32r, tag="wo")
            mask_sb = pers.tile([P, 896], f32r, tag="mask")
            ident_sb = pers.tile([P, P], f32r, tag="ident")
            ones_sb = pers.tile([65, 64], f32r, tag="ones")
            if use_kbias:
                kbias_sb = pers.tile([1, S], f32r, tag="kbias")
                ones_q = pers.tile([1, QC], f32r, tag="onesq")
                nc.sync.dma_start(out=kbias_sb, in_=kbias[:, :])
                nc.vector.memset(ones_q, 1.0)



            with (
                tc.tile_pool(name="pha", bufs=1) as pha,
                tc.tile_pool(name="xpool", bufs=3) as xpool,
                tc.tile_pool(name="projps", bufs=4, space="PSUM") as projps,
                tc.tile_pool(name="vps", bufs=2, space="PSUM") as vps,
            ):
                wq_sb = pha.tile([P, DIN_T, 256], f32r, tag="wq")
                wk_sb = pha.tile([P, DIN_T, 256], f32r, tag="wk")
                wv_sb = pha.tile([P, DIN_T, 256], f32r, tag="wv")
                c_sb = pha.tile([P, S], mybir.dt.float16, tag="ctab")
                t_sb = pha.tile([P, S], mybir.dt.float16, tag="ttab")

                for sc in range(NQC):
                    xTc = xpool.tile([P, DIN_T, QC], f32r, tag="xTc")
                    for kc in range(DIN_T):
                        nc.sync.dma_start(
                            out=xTc[:, kc, :],
                            in_=xT[kc * P:(kc + 1) * P, sc * QC:(sc + 1) * QC],
                        )
                        if sc == 0:
                            nc.sync.dma_start(out=wq_sb[:, kc, :], in_=wq[kc * P:(kc + 1) * P, :])
                            nc.sync.dma_start(out=wk_sb[:, kc, :], in_=wk[kc * P:(kc + 1) * P, :])
                            nc.sync.dma_start(out=wv_sb[:, kc, :], in_=wv[kc * P:(kc + 1) * P, :])
                    if sc == 0:
                        nc.sync.dma_start(out=c_sb, in_=ctab[:, :])
                        nc.sync.dma_start(out=t_sb, in_=ttab[:, :])
                    # q/k projections + RoPE for this s-chunk
                    cs = c_sb[:, sc * QC:(sc + 1) * QC]
                    ts = t_sb[:, sc * QC:(sc + 1) * QC]
                    for dst, w_sb in ((qT_sb, wq_sb), (kT_sb, wk_sb)):
                        for X in range(2):
                            pq = projps.tile([P, QC], f32, tag="pq")
                            for kc in range(DIN_T):
                                nc.tensor.matmul(
                                    pq,
                                    w_sb[:, kc, X * P:(X + 1) * P],
                                    xTc[:, kc, :],
                                    start=(kc == 0),
                                    stop=(kc == DIN_T - 1),
                                )
                            tmp = ropet.tile([P, QC], f32, tag="tmp")
                            z = ropet.tile([P, QC], mybir.dt.float16, tag="z")
                            zs = ropet.tile([P, QC], mybir.dt.float16, tag="zs")
                            nc.vector.tensor_mul(tmp, pq, cs)
                            nc.vector.tensor_mul(z, pq, ts)
                            for blk in (0, 64):
                                nc.sync.dma_start(out=zs[blk:blk + 32, :], in_=z[blk + 32:blk + 64, :])
                                nc.sync.dma_start(out=zs[blk + 32:blk + 64, :], in_=z[blk:blk + 32, :])
                            dv = dst[:, X, sc * QC:(sc + 1) * QC]
                            nc.vector.tensor_add(dv, tmp, zs)
                    # v projection for the 4 s-tiles of this chunk
                    for j in range(4):
                        st = sc * 4 + j
                        pv = vps.tile([P, 256], f32, tag="pv")
                        for kc in range(DIN_T):
                            nc.tensor.matmul(
                                pv,
                                xTc[:, kc, j * P:(j + 1) * P],
                                wv_sb[:, kc, :],
                                start=(kc == 0),
                                stop=(kc == DIN_T - 1),
                            )
                        nc.scalar.copy(
                            out=v_sb[:, :, st, 0:64],
                            in_=pv.rearrange("p (h c) -> p h c", h=HPC),
                        )

            for h in range(HPC):
                nc.sync.dma_start(out=wo_sb[:, h, :], in_=wo[h, :, :])
            nc.sync.dma_start(out=mask_sb, in_=mask[:, :])
            nc.sync.dma_start(out=ident_sb, in_=ident[:, :])
            nc.sync.dma_start(out=ones_sb, in_=ones_in[:, :])
            for h in range(HPC):
                nc.sync.dma_start(out=v_sb[:, h, :, 64:65], in_=vones.rearrange("p (k o) -> p k o", o=1))
            # ---- attention + fused out-projection (qc-outer) ----
            with (
                tc.tile_pool(name="epool", bufs=2) as epool,
                tc.tile_pool(name="rpool", bufs=1) as rpool,
                tc.tile_pool(name="ypool", bufs=3) as ypool,
                tc.tile_pool(name="scps", bufs=2, space="PSUM") as scps,
                tc.tile_pool(name="pvps", bufs=2, space="PSUM") as pvps,
                tc.tile_pool(name="bcps", bufs=1, space="PSUM") as bcps,
                tc.tile_pool(name="yps", bufs=1, space="PSUM") as yps,
            ):
                for qc in range(NQC):
                    q0 = qc * QC
                    nkt = q0 // P + 4 if is_causal else NKT
                    npair = (nkt + 1) // 2
                    for h in range(HPC):
                        X, o = h // 2, 64 * (h % 2)
                        qh = qT_sb[o:o + 64, X, :]
                        kh = kT_sb[o:o + 64, X, :]
                        O = pvps.tile([65, QC], f32, tag="O")
                        for pr in range(npair):
                            sc2 = scps.tile([P, 2 * QC], f32, tag="sc2")
                            e2 = epool.tile([P, 2 * QC], f32r, tag="e2")
                            for half in range(2):
                                kt = 2 * pr + half
                                if kt >= nkt:
                                    continue
                                scv = sc2[:, half * QC:(half + 1) * QC]
                                diag = is_causal and kt * P >= q0
                                nc.tensor.matmul(
                                    scv,
                                    kh[:, kt * P:(kt + 1) * P],
                                    qh[:, q0:q0 + QC],
                                    start=True,
                                    stop=not (diag or use_kbias),
                                )
                                if use_kbias:
                                    nc.tensor.matmul(
                                        scv,
                                        kbias_sb[:, kt * P:(kt + 1) * P],
                                        ones_q,
                                        start=False,
                                        stop=not diag,
                                    )
                                if diag:
                                    d = kt * P - q0
                                    nc.tensor.matmul(
                                        scv,
                                        ident_sb,
                                        mask_sb[:, 384 - d:896 - d],
                                        start=False,
                                        stop=True,
                                    )
                            nc.scalar.activation(out=e2, in_=sc2, func=EXP, scale=0.125)
                            for half in range(2):
                                kt = 2 * pr + half
                                if kt >= nkt:
                                    continue
                                nc.tensor.matmul(
                                    O,
                                    v_sb[:, h, kt, :],
                                    e2[:, half * QC:(half + 1) * QC],
                                    start=(kt == 0),
                                    stop=(kt == nkt - 1),
                                )
                        at = attnT_sb[0:64, h, q0:q0 + QC]
                        nc.vector.tensor_copy(at, O[0:64, :])
                        r = rpool.tile([65, QC], f32r, tag="r")
                        nc.vector.reciprocal(r[64:65, :], O[64:65, :])
                        bc = bcps.tile([64, QC], f32, tag="bc")
                        nc.tensor.matmul(
                            bc, ones_sb[64:65, :], r[64:65, :],
                            start=True, stop=True,
                        )
                        nc.vector.tensor_mul(at, at, bc)
                    # out projection for this q-chunk's four s-tiles
                    for j in range(4):
                        st = qc * 4 + j
                        for nb in range(2):
                            yp = yps.tile([P, QC], f32, tag="yp")
                            for h in range(HPC):
                                nc.tensor.matmul(
                                    yp,
                                    attnT_sb[:, h, st * P:(st + 1) * P],
                                    wo_sb[:, h, nb * QC:(nb + 1) * QC],
                                    start=(h == 0),
                                    stop=(h == HPC - 1),
                                )
                            yt = ypool.tile([P, QC], f32, tag="yt")
                            if nb == 0:
                                nc.vector.tensor_copy(yt, yp)
                            else:
                                nc.scalar.copy(out=yt, in_=yp)
                            nc.sync.dma_start(
                                out=y[st * P:(st + 1) * P, nb * QC:(nb + 1) * QC], in_=yt
                            )

    _split_matmul_waits(nc, wfix_sem)
    return nc


def _split_matmul_waits(nc, wfix_sem):
    """Walrus's engine-instruction sync-wait slots are scarce (fp32r matmul
    takes exactly one; DVE/ACT structs also cap out). Leave one wait on the
    instruction and move the rest onto NoOps inserted just before it, each
    carrying a single wait."""
    import concourse.mybir as mybir
    import bass_rust

    n_fix = 0
    for blk in nc.m.functions[0].blocks:
        il = blk.instructions
        out = []
        changed = False
        for inst in il:
            si = inst.sync_info
            if si is not None and len(si.on_wait) > 1:
                merged = {}
                for w in si.on_wait:
                    k = (w.sync_type, w.id, w.wait_mode)
                    if (
                        k in merged
                        and w.wait_mode == "sem-ge-imm"
                        and w.wait_reg is None
                    ):
                        if w.wait_value > merged[k].wait_value:
                            merged[k] = w
                    elif k in merged:
                        merged[(k, len(merged))] = w
                    else:
                        merged[k] = w
                waits = list(merged.values())
                if len(waits) == 1:
                    si.on_wait = waits
                    out.append(inst)
                    continue
                for j, w in enumerate(waits[:-1]):
                    nop = mybir.InstNoOp(name=f"{inst.name}-wfix{j}")
                    nop.engine = inst.engine
                    upd = bass_rust.SyncUpdate(
                        sync_type="semaphore", id=wfix_sem.num,
                        ant_name=wfix_sem.name, update_mode="sem-inc",
                        update_value=1, update_reg=None,
                    )
                    nop.sync_info = bass_rust.SyncInfo(on_wait=[w], on_update=[upd])
                    out.append(nop)
                    n_fix += 1
                si.on_wait = [waits[-1]]
                changed = True
            out.append(inst)
        if changed:
            blk.instructions = out


def _host_tables():
    j = np.arange(32)
    inv_freq = (10000.0 ** (-j / 32.0)).astype(np.float64)
    ang = np.arange(S, dtype=np.float64)[:, None] * inv_freq[None, :]  # [S, 32]
    cosv = np.cos(ang).astype(np.float32).T   # [32, S]
    sinv = np.sin(ang).astype(np.float32).T
    C = np.empty((P, S), dtype=np.float32)
    T = np.empty((P, S), dtype=np.float32)
    for blk in (0, 64):
        C[blk:blk + 32] = cosv
        C[blk + 32:blk + 64] = cosv
        T[blk:blk + 32] = sinv          # lo rows carry +sin (headed to hi output)
        T[blk + 32:blk + 64] = -sinv    # hi rows carry -sin (headed to lo output)
    i = np.arange(P)[:, None]
    u = np.arange(896)[None, :]
    M = np.where(u >= i + 384, 0.0, MBIG).astype(np.float32)
    return C.astype(np.float16), T.astype(np.float16), M


def _in_maps(x, qkv_w, out_w, attn_mask, is_causal):
    C, T, M = _host_tables()
    ident = np.eye(P, dtype=np.float32)
    wq_full = qkv_w[:, 0:D]
    wk_full = qkv_w[:, D:2 * D]
    wv_full = qkv_w[:, 2 * D:3 * D]
    use_kbias = (not is_causal) and not bool(np.all(attn_mask))
    maps = []
    for core in range(8):
        b, hg = core // 4, core % 4
        cols = slice(hg * 256, (hg + 1) * 256)
        if use_kbias:
            kb = np.where(attn_mask[b], 0.0, MBIG).astype(np.float32)[None, :]
        else:
            kb = np.zeros((1, S), dtype=np.float32)
        maps.append(
            dict(
                xT=np.ascontiguousarray(x[b].T),
                wq=np.ascontiguousarray(wq_full[:, cols]),
                wk=np.ascontiguousarray(wk_full[:, cols]),
                wv=np.ascontiguousarray(wv_full[:, cols]),
                wo=np.ascontiguousarray(
                    out_w[hg * 256:(hg + 1) * 256, :].reshape(HPC, 64, D)
                ),
                ones_in=np.ones((65, 64), dtype=np.float32),
                vones=np.ones((P, NKT), dtype=np.float32),
                ctab=C,
                ttab=T,
                mask=M,
                ident=ident,
                kbias=kb,
            )
        )
    return maps, use_kbias


def kernel(x, qkv_w, out_w, attn_mask, is_causal):
    from concourse.bass_utils import run_bass_kernel_spmd

    x = np.asarray(x, dtype=np.float32)
    qkv_w = np.asarray(qkv_w, dtype=np.float32)
    out_w = np.asarray(out_w, dtype=np.float32)
    attn_mask = np.asarray(attn_mask).astype(bool)
    causal = bool(np.asarray(is_causal))

    maps, use_kbias = _in_maps(x, qkv_w, out_w, attn_mask, causal)
    nc = _build_nc(causal, use_kbias)
    res = run_bass_kernel_spmd(nc, maps, list(range(8)))
    out = np.zeros((2, S, D), dtype=np.float32)
    for core in range(8):
        out[core // 4] += res.results[core]["y"]
    return out



# revision 10
# speedup vs baseline: 1.4129x; 1.4129x over previous
"""Fused causal attention block (qkv proj + RoPE + attention + out proj) on 8 TRN2 cores.

Sharding: data-parallel over batch (2) x tensor-parallel over heads (16 -> 4 per core).
Each core computes y_partial[b] = attn_heads_group(x[b]) @ out_w[group_rows]; the host
sums the 4 partials per batch (the out-projection "all-reduce") and stacks batches.

Device kernel (per core, Tile framework), bf16 matmul datapath:
  phase A: batched-DMA xT/weights (bf16), compute qT,kT ([d_head, s] layout) with RoPE.
           The rotate-half partition swap is a PE permutation matmul (PERM @ z) instead
           of SBUF-to-SBUF DMAs; v is produced in natural [s, d_head] layout with a
           ones-column so the softmax denominator falls out of the PV matmul.
  phase B: per head, per 512-wide q chunk: scores^T = k_tile^T @ q into PSUM; the causal
           diagonal block is trimmed to widths 512/384/256/128 with a single reusable
           [128,128] triangular bias injected via identity matmul at N=128; exp on ACT
           (scale=1/8, no max subtraction - logits are O(5) by construction); PV
           accumulation; normalization via DVE reciprocal + GPSIMD partition_broadcast +
           DVE multiply. Heads are packed in pairs for the out-projection (contraction
           128); odd heads hop to partitions 64-127 via one small SBUF DMA.
  phase C: y = attnT^T @ wo per 128-row s-tile (2 matmuls, K=256 over paired heads),
           copied to SBUF and DMA'd out as one [128, 1024] transfer per s-tile.
"""

import numpy as np

S = 2048
D = 1024
H = 16
DH = 64
P = 128
HPC = 4          # heads per core
QC = 512         # q-chunk width
NQC = S // QC
NKT = S // P     # k tiles
DIN_T = D // P   # contraction tiles for projections
NST = S // P     # s tiles
MBIG = -239616.0  # pre-scale mask bias (bf16-exact); * 0.125 = -29952 -> exp == 0.0


def _build_nc(is_causal: bool, use_kbias: bool):
    import concourse.bass as bass
    import concourse.mybir as mybir
    import concourse.tile as tile

    f32 = mybir.dt.float32
    f32r = mybir.dt.float32r
    bf16 = mybir.dt.bfloat16
    f16 = mybir.dt.float16
    EXP = mybir.ActivationFunctionType.Exp

    nc = bass.Bass()
    wfix_sem = nc.alloc_semaphore("wfix")
    xT = nc.dram_tensor("xT", [P, DIN_T, S], bf16, kind="ExternalInput")
    wq = nc.dram_tensor("wq", [P, DIN_T, 256], bf16, kind="ExternalInput")
    wk = nc.dram_tensor("wk", [P, DIN_T, 256], bf16, kind="ExternalInput")
    wv = nc.dram_tensor("wv", [P, DIN_T, 256], bf16, kind="ExternalInput")
    wo = nc.dram_tensor("wo", [P, 2, D], bf16, kind="ExternalInput")
    ctab = nc.dram_tensor("ctab", [P, S], f16, kind="ExternalInput")
    ttab = nc.dram_tensor("ttab", [P, S], f16, kind="ExternalInput")
    tri = nc.dram_tensor("tri", [P, P], bf16, kind="ExternalInput")
    ident = nc.dram_tensor("ident", [P, P], bf16, kind="ExternalInput")
    perm = nc.dram_tensor("perm", [P, P], bf16, kind="ExternalInput")
    kbias = nc.dram_tensor("kbias", [1, S], bf16, kind="ExternalInput")
    y = nc.dram_tensor("y", [S, D], f32, kind="ExternalOutput")

    with tile.TileContext(nc) as tc, nc.allow_low_precision(
        reason="bf16 matmul datapath; fp32 PSUM accumulation keeps the error "
        "well inside the 2e-2 harness tolerance"
    ):
        with (
            tc.tile_pool(name="pers", bufs=1) as pers,
            tc.tile_pool(name="ropet", bufs=3) as ropet,
        ):
            qT_sb = pers.tile([P, 2, S], bf16, tag="qT")
            kT_sb = pers.tile([P, 2, S], bf16, tag="kT")
            v_sb = pers.tile([P, HPC, NKT, 65], bf16, tag="v")
            attnT_sb = pers.tile([P, 2, S], bf16, tag="attnT")
            wo_sb = pers.tile([P, 2, D], bf16, tag="wo")
            tri_sb = pers.tile([P, P], bf16, tag="tri")
            ident_sb = pers.tile([P, P], bf16, tag="ident")
            perm_sb = pers.tile([P, P], bf16, tag="perm")
            c_sb = pers.tile([P, S], f16, tag="ctab")
            t_sb = pers.tile([P, S], f16, tag="ttab")
            ones_sb = pers.tile([65, 64], f32r, tag="ones")
            if use_kbias:
                kbias_sb = pers.tile([1, S], bf16, tag="kbias")
                ones_q = pers.tile([1, QC], bf16, tag="onesq")
                nc.sync.dma_start(out=kbias_sb, in_=kbias[:, :])

            with (
                tc.tile_pool(name="pha", bufs=1) as pha,
                tc.tile_pool(name="xpool", bufs=3) as xpool,
                tc.tile_pool(name="projps", bufs=2, space="PSUM") as projps,
                tc.tile_pool(name="zsps", bufs=2, space="PSUM") as zsps,
                tc.tile_pool(name="vps", bufs=2, space="PSUM") as vps,
            ):
                wq_sb = pha.tile([P, DIN_T, 256], bf16, tag="wq")
                wk_sb = pha.tile([P, DIN_T, 256], bf16, tag="wk")
                wv_sb = pha.tile([P, DIN_T, 256], bf16, tag="wv")

                # issue order matters: the first q-proj matmul needs only the
                # first half of wq and of the chunk-0 x tile.
                nc.sync.dma_start(out=wq_sb[:, 0:4, :], in_=wq[:, 0:4, :])

                for sc in range(NQC):
                    xTc = xpool.tile([P, DIN_T, QC], bf16, tag="xTc")
                    for hh in range(2):
                        nc.sync.dma_start(
                            out=xTc[:, hh * 4:(hh + 1) * 4, :],
                            in_=xT[:, hh * 4:(hh + 1) * 4, sc * QC:(sc + 1) * QC],
                        )
                        if sc == 0 and hh == 0:
                            nc.sync.dma_start(out=wq_sb[:, 4:8, :], in_=wq[:, 4:8, :])
                            nc.sync.dma_start(out=c_sb, in_=ctab[:, :])
                            nc.sync.dma_start(out=t_sb, in_=ttab[:, :])
                            # constant-1 fills (walrus rejects 16-bit memsets):
                            # out = Copy(c * 0 + 1); c_sb is loaded by then.
                            COPY = mybir.ActivationFunctionType.Copy
                            nc.scalar.activation(
                                out=v_sb[:, :, :, 64:65],
                                in_=c_sb[:, 0:64].rearrange(
                                    "p (a b o) -> p a b o", a=HPC, o=1
                                ),
                                func=COPY, scale=0.0, bias=1.0,
                            )
                            nc.scalar.activation(
                                out=ones_sb[64:65, :], in_=c_sb[64:65, 0:64],
                                func=COPY, scale=0.0, bias=1.0,
                            )
                            if use_kbias:
                                nc.scalar.activation(
                                    out=ones_q, in_=c_sb[0:1, 0:QC],
                                    func=COPY, scale=0.0, bias=1.0,
                                )
                    if sc == 0:
                        nc.sync.dma_start(out=perm_sb, in_=perm[:, :])
                        nc.sync.dma_start(out=wk_sb, in_=wk[:, :, :])
                        nc.sync.dma_start(out=wv_sb, in_=wv[:, :, :])
                    elif sc == 1:
                        nc.sync.dma_start(out=wo_sb, in_=wo[:, :, :])
                        nc.sync.dma_start(out=tri_sb, in_=tri[:, :])
                        nc.sync.dma_start(out=ident_sb, in_=ident[:, :])
                    # q/k projections + RoPE for this s-chunk
                    cs = c_sb[:, sc * QC:(sc + 1) * QC]
                    ts = t_sb[:, sc * QC:(sc + 1) * QC]
                    for dst, w_sb in ((qT_sb, wq_sb), (kT_sb, wk_sb)):
                        for X in range(2):
                            pq = projps.tile([P, QC], f32, tag="pq")
                            for kc in range(DIN_T):
                                nc.tensor.matmul(
                                    pq,
                                    w_sb[:, kc, X * P:(X + 1) * P],
                                    xTc[:, kc, :],
                                    start=(kc == 0),
                                    stop=(kc == DIN_T - 1),
                                )
                            tmp = ropet.tile([P, QC], bf16, tag="tmp")
                            z = ropet.tile([P, QC], bf16, tag="z")
                            nc.vector.tensor_mul(tmp, pq, cs)
                            nc.vector.tensor_mul(z, pq, ts)
                            zsp = zsps.tile([P, QC], f32, tag="zs")
                            nc.tensor.matmul(zsp, perm_sb, z, start=True, stop=True)
                            dv = dst[:, X, sc * QC:(sc + 1) * QC]
                            nc.vector.tensor_add(dv, tmp, zsp)
                    # v projection for the 4 s-tiles of this chunk
                    for j in range(4):
                        st = sc * 4 + j
                        pv = vps.tile([P, 256], f32, tag="pv")
                        for kc in range(DIN_T):
                            nc.tensor.matmul(
                                pv,
                                xTc[:, kc, j * P:(j + 1) * P],
                                wv_sb[:, kc, :],
                                start=(kc == 0),
                                stop=(kc == DIN_T - 1),
                            )
                        nc.scalar.copy(
                            out=v_sb[:, :, st, 0:64],
                            in_=pv.rearrange("p (h c) -> p h c", h=HPC),
                        )

            # ---- attention + fused out-projection (qc-outer) ----
            with (
                tc.tile_pool(name="epool", bufs=3) as epool,
                tc.tile_pool(name="rpool", bufs=2) as rpool,
                tc.tile_pool(name="bcpool", bufs=2) as bcpool,
                tc.tile_pool(name="atodd", bufs=2) as atodd,
                tc.tile_pool(name="ypool", bufs=2) as ypool,
                tc.tile_pool(name="scps", bufs=2, space="PSUM") as scps,
                tc.tile_pool(name="pvps", bufs=2, space="PSUM") as pvps,
                tc.tile_pool(name="yps", bufs=2, space="PSUM") as yps,
            ):
                for qc in range(NQC):
                    q0 = qc * QC
                    nkt = 4 * qc + 4 if is_causal else NKT
                    nfull = 4 * qc if is_causal else NKT
                    # odd heads first: their attnT partition-hop DMA overlaps
                    # the remaining heads' compute.
                    for h in (1, 3, 0, 2):
                        X, o = h // 2, 64 * (h % 2)
                        qh = qT_sb[o:o + 64, X, :]
                        kh = kT_sb[o:o + 64, X, :]
                        O = pvps.tile([65, QC], f32, tag="O")
                        for pr in range(nfull // 2):
                            sc2 = scps.tile([P, 2 * QC], f32, tag="sc2")
                            for half in range(2):
                                kt = 2 * pr + half
                                scv = sc2[:, half * QC:(half + 1) * QC]
                                nc.tensor.matmul(
                                    scv,
                                    kh[:, kt * P:(kt + 1) * P],
                                    qh[:, q0:q0 + QC],
                                    start=True,
                                    stop=not use_kbias,
                                )
                                if use_kbias:
                                    nc.tensor.matmul(
                                        scv,
                                        kbias_sb[:, kt * P:(kt + 1) * P],
                                        ones_q,
                                        start=False,
                                        stop=True,
                                    )
                            e2 = epool.tile([P, 2 * QC], bf16, tag="e2")
                            nc.scalar.activation(out=e2, in_=sc2, func=EXP, scale=0.125)
                            for half in range(2):
                                kt = 2 * pr + half
                                nc.tensor.matmul(
                                    O,
                                    v_sb[:, h, kt, :],
                                    e2[:, half * QC:(half + 1) * QC],
                                    start=(kt == 0),
                                    stop=(kt == nkt - 1),
                                )
                        if is_causal:
                            # diagonal block: k-tiles base..base+3, trimmed to the
                            # causal lower-triangle at 128-column granularity; the
                            # within-tile triangle bias is injected first (start=True)
                            # so the score matmul closes the accumulation group.
                            base = 4 * qc
                            D1 = scps.tile([P, 2 * QC], f32, tag="sc2")
                            nc.tensor.matmul(D1[:, 0:128], ident_sb, tri_sb,
                                             start=True, stop=False)
                            nc.tensor.matmul(
                                D1[:, 0:512],
                                kh[:, (base + 0) * P:(base + 1) * P],
                                qh[:, q0:q0 + 512],
                                start=False, stop=True,
                            )
                            nc.tensor.matmul(D1[:, 512:640], ident_sb, tri_sb,
                                             start=True, stop=False)
                            nc.tensor.matmul(
                                D1[:, 512:896],
                                kh[:, (base + 1) * P:(base + 2) * P],
                                qh[:, q0 + 128:q0 + 512],
                                start=False, stop=True,
                            )
                            e2a = epool.tile([P, 2 * QC], bf16, tag="e2")
                            nc.scalar.activation(out=e2a[:, 0:896], in_=D1[:, 0:896],
                                                 func=EXP, scale=0.125)
                            nc.tensor.matmul(O, v_sb[:, h, base + 0, :],
                                             e2a[:, 0:512],
                                             start=(base == 0), stop=False)
                            nc.tensor.matmul(O[:, 128:512], v_sb[:, h, base + 1, :],
                                             e2a[:, 512:896],
                                             start=False, stop=False)
                            D2 = scps.tile([P, 2 * QC], f32, tag="sc2")
                            nc.tensor.matmul(D2[:, 0:128], ident_sb, tri_sb,
                                             start=True, stop=False)
                            nc.tensor.matmul(
                                D2[:, 0:256],
                                kh[:, (base + 2) * P:(base + 3) * P],
                                qh[:, q0 + 256:q0 + 512],
                                start=False, stop=True,
                            )
                            nc.tensor.matmul(D2[:, 256:384], ident_sb, tri_sb,
                                             start=True, stop=False)
                            nc.tensor.matmul(
                                D2[:, 256:384],
                                kh[:, (base + 3) * P:(base + 4) * P],
                                qh[:, q0 + 384:q0 + 512],
                                start=False, stop=True,
                            )
                            e2b = epool.tile([P, 2 * QC], bf16, tag="e2")
                            nc.scalar.activation(out=e2b[:, 0:384], in_=D2[:, 0:384],
                                                 func=EXP, scale=0.125)
                            nc.tensor.matmul(O[:, 256:512], v_sb[:, h, base + 2, :],
                                             e2b[:, 0:256],
                                             start=False, stop=False)
                            nc.tensor.matmul(O[:, 384:512], v_sb[:, h, base + 3, :],
                                             e2b[:, 256:384],
                                             start=False, stop=True)
                        # normalize: recip on the denominator row, K=1 broadcast
                        # matmul across partitions, copy + in-place multiply on DVE
                        # (TensorTensor may read only one PSUM operand).
                        r = rpool.tile([65, QC], f32r, tag="r")
                        nc.vector.reciprocal(r[64:65, :], O[64:65, :])
                        bcv = yps.tile([64, QC], f32, tag="yp")
                        nc.tensor.matmul(bcv, ones_sb[64:65, :], r[64:65, :],
                                         start=True, stop=True)
                        if h % 2 == 0:
                            at = attnT_sb[0:64, X, q0:q0 + QC]
                            nc.vector.tensor_copy(at, O[0:64, :])
                            nc.vector.tensor_mul(at, at, bcv)
                        else:
                            ato = atodd.tile([64, QC], bf16, tag="ato")
                            nc.vector.tensor_copy(ato, O[0:64, :])
                            nc.vector.tensor_mul(ato, ato, bcv)
                            nc.sync.dma_start(
                                out=attnT_sb[64:128, X, q0:q0 + QC], in_=ato
                            )
                    # out projection for this q-chunk's four s-tiles (paired heads)
                    for j in range(4):
                        st = qc * 4 + j
                        yt = ypool.tile([P, 2 * QC], f32, tag="yt")
                        for nb in range(2):
                            yp = yps.tile([P, QC], f32, tag="yp")
                            for g in range(2):
                                nc.tensor.matmul(
                                    yp,
                                    attnT_sb[:, g, st * P:(st + 1) * P],
                                    wo_sb[:, g, nb * QC:(nb + 1) * QC],
                                    start=(g == 0),
                                    stop=(g == 1),
                                )
                            nc.vector.tensor_copy(yt[:, nb * QC:(nb + 1) * QC], yp)
                        nc.sync.dma_start(out=y[st * P:(st + 1) * P, :], in_=yt)

    _split_matmul_waits(nc, wfix_sem)
    return nc


def _split_matmul_waits(nc, wfix_sem):
    """Walrus's engine-instruction sync-wait slots are scarce (matmul takes
    exactly one; DVE/ACT structs also cap out). Leave one wait on the
    instruction and move the rest onto NoOps inserted just before it, each
    carrying a single wait."""
    import concourse.mybir as mybir
    import bass_rust

    n_fix = 0
    for blk in nc.m.functions[0].blocks:
        il = blk.instructions
        out = []
        changed = False
        for inst in il:
            si = inst.sync_info
            if si is not None and len(si.on_wait) > 1:
                merged = {}
                for w in si.on_wait:
                    k = (w.sync_type, w.id, w.wait_mode)
                    if (
                        k in merged
                        and w.wait_mode == "sem-ge-imm"
                        and w.wait_reg is None
                    ):
                        if w.wait_value > merged[k].wait_value:
                            merged[k] = w
                    elif k in merged:
                        merged[(k, len(merged))] = w
                    else:
                        merged[k] = w
                waits = list(merged.values())
                if len(waits) == 1:
                    si.on_wait = waits
                    out.append(inst)
                    continue
                for j, w in enumerate(waits[:-1]):
                    nop = mybir.InstNoOp(name=f"{inst.name}-wfix{j}")
                    nop.engine = inst.engine
                    upd = bass_rust.SyncUpdate(
                        sync_type="semaphore", id=wfix_sem.num,
                        ant_name=wfix_sem.name, update_mode="sem-inc",
                        update_value=1, update_reg=None,
                    )
                    nop.sync_info = bass_rust.SyncInfo(on_wait=[w], on_update=[upd])
                    out.append(nop)
                    n_fix += 1
                si.on_wait = [waits[-1]]
                changed = True
            out.append(inst)
        if changed:
            blk.instructions = out


def _host_tables():
    j = np.arange(32)
    inv_freq = (10000.0 ** (-j / 32.0)).astype(np.float64)
    ang = np.arange(S, dtype=np.float64)[:, None] * inv_freq[None, :]  # [S, 32]
    cosv = np.cos(ang).astype(np.float32).T   # [32, S]
    sinv = np.sin(ang).astype(np.float32).T
    C = np.empty((P, S), dtype=np.float32)
    T = np.empty((P, S), dtype=np.float32)
    for blk in (0, 64):
        C[blk:blk + 32] = cosv
        C[blk + 32:blk + 64] = cosv
        T[blk:blk + 32] = sinv          # lo rows carry +sin (headed to hi output)
        T[blk + 32:blk + 64] = -sinv    # hi rows carry -sin (headed to lo output)
    i = np.arange(P)[:, None]
    u = np.arange(P)[None, :]
    TRI = np.where(u >= i, 0.0, MBIG).astype(np.float32)
    # rotate-half partition swap: blocks (0-31 <-> 32-63), (64-95 <-> 96-127)
    swap = np.arange(P)
    swap = (swap // 64) * 64 + ((swap % 64) + 32) % 64
    PERM = np.zeros((P, P), dtype=np.float32)
    PERM[swap, np.arange(P)] = 1.0
    return C.astype(np.float16), T.astype(np.float16), TRI, PERM


def _in_maps(x, qkv_w, out_w, attn_mask, is_causal):
    import ml_dtypes

    bf16 = ml_dtypes.bfloat16
    C, T, TRI, PERM = _host_tables()
    ident = np.eye(P, dtype=np.float32)
    wq_full = qkv_w[:, 0:D]
    wk_full = qkv_w[:, D:2 * D]
    wv_full = qkv_w[:, 2 * D:3 * D]
    use_kbias = (not is_causal) and not bool(np.all(attn_mask))

    def wlayout(w):
        # [D, 256] -> [128, 8, 256] with [p, kc, :] = w[kc*128 + p, :]
        return np.ascontiguousarray(
            w.reshape(DIN_T, P, 256).transpose(1, 0, 2).astype(bf16)
        )

    maps = []
    for core in range(8):
        b, hg = core // 4, core % 4
        cols = slice(hg * 256, (hg + 1) * 256)
        if use_kbias:
            kb = np.where(attn_mask[b], 0.0, MBIG).astype(np.float32)[None, :]
        else:
            kb = np.zeros((1, S), dtype=np.float32)
        xTb = np.ascontiguousarray(x[b].T)  # [D, S]
        xTb = np.ascontiguousarray(
            xTb.reshape(DIN_T, P, S).transpose(1, 0, 2).astype(bf16)
        )
        # wo pairs: [128, 2, D]; rows 0-63 = head 2g, 64-127 = head 2g+1
        wob = out_w[hg * 256:(hg + 1) * 256, :].reshape(2, 2, 64, D)
        wob = np.ascontiguousarray(wob.transpose(1, 2, 0, 3).reshape(P, 2, D).astype(bf16))
        maps.append(
            dict(
                xT=xTb,
                wq=wlayout(wq_full[:, cols]),
                wk=wlayout(wk_full[:, cols]),
                wv=wlayout(wv_full[:, cols]),
                wo=wob,
                ctab=C,
                ttab=T,
                tri=TRI.astype(bf16),
                ident=ident.astype(bf16),
                perm=PERM.astype(bf16),
                kbias=kb.astype(bf16),
            )
        )
    return maps, use_kbias


def kernel(x, qkv_w, out_w, attn_mask, is_causal):
    from concourse.bass_utils import run_bass_kernel_spmd

    x = np.asarray(x, dtype=np.float32)
    qkv_w = np.asarray(qkv_w, dtype=np.float32)
    out_w = np.asarray(out_w, dtype=np.float32)
    attn_mask = np.asarray(attn_mask).astype(bool)
    causal = bool(np.asarray(is_causal))

    maps, use_kbias = _in_maps(x, qkv_w, out_w, attn_mask, causal)
    nc = _build_nc(causal, use_kbias)
    res = run_bass_kernel_spmd(nc, maps, list(range(8)))
    out = np.zeros((2, S, D), dtype=np.float32)
    for core in range(8):
        out[core // 4] += res.results[core]["y"]
    return out
